# revision 11
# baseline (speedup 1.0000x reference)
"""Chamfer distance kernel for Trainium2, batch-parallel across 8 NeuronCores.

Reference computation (per batch b, points a=input1[b] [N,3], bb=input2[b] [M,3]):
    d[n,m]  = |a_n - b_m|^2 (clamped >= 0)
    dist0_n = min_m d[n,m];  dist1_m = min_n d[n,m]
    loss_b  = max(mean_n sqrt(dist0), mean_m sqrt(dist1));  out = mean_b loss_b

Device strategy (per core: 4 batches, two symmetric passes for dist0/dist1):
  * d[n,m] = a2[n] + b2[m] - 2 a.b is computed entirely on the PE as a K=24
    matmul: every fp32 factor is a 3-term bf16 split (~2^-27 relative), the
    rank-1 a2/b2 terms ride extra ones-rows. bf16 streams 1 col/cycle (fp32
    would be 4x slower); K only affects LDWEIGHTS which is hidden.
  * The 4 m-chunks of one 128-row tile are packed as 4 concurrent row-group
    matmuls (tile_position=(32g,0)) into 4 PSUM banks.
  * min over the free dim, split across two engine lanes per 16-tile group:
      - direct lane (4/16): DVE tensor_reduce(min) straight from PSUM
      - copy lane  (12/16): ACT copies PSUM->SBUF, then one fused DVE
        tensor_tensor_reduce(min,min) over the two SBUF halves consumes
        2 elem/lane/cycle.
  * Raw per-row minima (= dist0/dist1 already including a2/b2) go back to the
    host, which does the exact scalar tail: clamp, sqrt, means, max, mean.
"""

import numpy as np
import ml_dtypes

import concourse.bacc as bacc
import concourse.mybir as mybir
import concourse.tile as tile
from concourse.bass_utils import run_bass_kernel_spmd
from concourse.dve_spec import Spec, Src0, Src1, C0, minn, lower as _dve_lower, _has_src1
from concourse.dve_ops import DveOp, OPS, _SUB_OPCODE_FOR_NAME, CUSTOM_DVE_SPECS
from concourse.dve_uop import DveOpSpec

BF16 = np.dtype(ml_dtypes.bfloat16)


def _register_min_reduce():
    """Runtime-register a fused custom DVE op: out=min(in0,in1) elementwise,
    accum_out=min-reduce of out (init s0). Streams 2 elems/lane/cycle from
    SBUF via both read ports — 2x a plain tensor_reduce."""
    name = "TT_MIN_REDUCE_ANT"
    if name in _SUB_OPCODE_FOR_NAME:
        return next(o for o in OPS if o.name == name)
    spec = Spec(body=minn(Src0, Src1), accum=minn, accum_init=C0)
    row = max(_SUB_OPCODE_FOR_NAME.values()) + 1
    _SUB_OPCODE_FOR_NAME[name] = row
    shas = {}
    for ver in ("v3", "v4"):
        s = DveOpSpec(name=name, opcode=row, uops=_dve_lower(spec, ver=ver),
                      rd1_en=_has_src1(spec))
        shas[ver] = s.sha(ver)
    op = DveOp(name, spec, subdim=False, uops_sha=shas)
    OPS.append(op)
    CUSTOM_DVE_SPECS[name] = spec
    return op


_MIN_OP = _register_min_reduce()

B, N, M, D = 32, 2048, 2048, 3
NCORES = 8
BPC = B // NCORES  # batches per core
P = 128            # output partitions per matmul tile
NT = N // P        # 16 n-tiles per batch
MJ = 512           # moving-operand free dim per matmul (one PSUM bank)
NG = M // MJ       # 4 row-group-packed matmuls per psum row-tile
K = 24             # contraction rows (18 coord cross-terms + 3 b2 + 3 a2)

_built_nc = None
last_results = None  # BassKernelResults of the most recent run (for test harness)
trace = False        # set True to capture an NTFF profile

FLT_BIG = 3.0e38


def _build():
    nc = bacc.Bacc("TRN2", target_bir_lowering=False, debug=False)
    lhsA = nc.dram_tensor("lhsA", [BPC, P, N], mybir.dt.bfloat16, kind="ExternalInput")
    rhsA = nc.dram_tensor("rhsA", [BPC, P, MJ], mybir.dt.bfloat16, kind="ExternalInput")
    lhsB = nc.dram_tensor("lhsB", [BPC, P, M], mybir.dt.bfloat16, kind="ExternalInput")
    rhsB = nc.dram_tensor("rhsB", [BPC, P, MJ], mybir.dt.bfloat16, kind="ExternalInput")
    outs = nc.dram_tensor("mins", [2, BPC, P, NT], mybir.dt.float32, kind="ExternalOutput")

    with tile.TileContext(nc) as tc:
        with (
            tc.tile_pool(name="ops", bufs=1) as ops,
            tc.tile_pool(name="psum", bufs=2, space="PSUM") as psum,
            tc.tile_pool(name="sb", bufs=4) as sbp,
            tc.tile_pool(name="res", bufs=2) as res,
        ):
            # warm the ACT Copy table (one-time ~2.7us load) while DMAs run
            warm = sbp.tile([P, 1], mybir.dt.float32, tag="warm")
            nc.gpsimd.memset(warm[:], 0.0)
            nc.scalar.copy(out=warm[:], in_=warm[:])
            # prefetch every operand tile up front (fits easily in SBUF)
            tiles = []
            for b in range(BPC):
                for pi, (lhs_d, rhs_d) in enumerate(((lhsA, rhsA), (lhsB, rhsB))):
                    lhs_t = ops.tile([P, N], mybir.dt.bfloat16, tag=f"lhs{b}_{pi}")
                    rhs_t = ops.tile([P, MJ], mybir.dt.bfloat16, tag=f"rhs{b}_{pi}")
                    nc.sync.dma_start(lhs_t[:], lhs_d[b])
                    nc.sync.dma_start(rhs_t[:], rhs_d[b])
                    tiles.append((b, pi, lhs_t, rhs_t))
            for b, pi, lhs_t, rhs_t in tiles:
                    mins_t = res.tile([P, NT], mybir.dt.float32, tag="mins")
                    for t in range(NT):
                        # L/R psum halves (2 banks each): R frees as soon as
                        # ACT has copied it out; L frees after the DVE fuse.
                        psl = psum.tile([P, M // 2], mybir.dt.float32, tag="psL")
                        psr = psum.tile([P, M // 2], mybir.dt.float32, tag="psR")
                        for g in range(NG):
                            dst = psl if g < 2 else psr
                            nc.tensor.matmul(
                                dst[:, (g % 2) * MJ:(g % 2 + 1) * MJ],
                                lhs_t[32 * g:32 * g + K, t * P:(t + 1) * P],
                                rhs_t[32 * g:32 * g + K, :],
                                start=True,
                                stop=True,
                                tile_position=(32 * g, 0),
                            )
                        # ACT evacuates the right half; DVE streams the left
                        # half from PSUM + the copied half from SBUF, fusing
                        # elementwise min with the min-reduction in one op.
                        sbh = sbp.tile([P, M // 2], mybir.dt.float32, tag="sbh")
                        nc.scalar.copy(out=sbh[:], in_=psr[:])
                        scratch = sbp.tile([P, M // 2], mybir.dt.float32, tag="scr")
                        nc.vector._custom_dve(
                            _MIN_OP,
                            out=scratch[:],
                            in0=psl[:],
                            in1=sbh[:],
                            s0=FLT_BIG,
                            accum_out=mins_t[:, t:t + 1],
                        )
                    nc.sync.dma_start(outs[pi, b], mins_t[:])
    nc.compile()
    return nc


def _get_nc():
    global _built_nc
    if _built_nc is None:
        _built_nc = _build()
    return _built_nc


def _split3(x64):
    """Split fp64 array into 3 bf16 terms summing to x to ~2^-27 relative."""
    h = x64.astype(BF16)
    r = x64 - h.astype(np.float64)
    m = r.astype(BF16)
    l = (r - m.astype(np.float64)).astype(BF16)
    return h, m, l


def _pack(s, t):
    """Operand rows so sum_k lhs[k,n] rhs[k,m] = |s_n|^2 + |t_m|^2 - 2 s_n . t_m.

    s, t: [BPC, N, 3] float32. Returns (lhs [BPC,128,N], rhs [BPC,128,MJ]) bf16
    with the K=24 rows replicated into 4 row-groups of 32 partitions; row-group
    g's rhs carries m-chunk [512g, 512g+512).
    """
    sT = np.ascontiguousarray(s.transpose(0, 2, 1)).astype(np.float64)        # [BPC,3,N]
    tT = np.ascontiguousarray(-2.0 * t.transpose(0, 2, 1)).astype(np.float64)  # [BPC,3,M]
    sh, sm, sl = _split3(sT)
    th, tm, tl = _split3(tT)
    t2 = np.sum(t.astype(np.float64) ** 2, axis=2)           # [BPC, M]
    s2 = np.sum(s.astype(np.float64) ** 2, axis=2)           # [BPC, N]
    t2h, t2m, t2l = _split3(t2)
    s2h, s2m, s2l = _split3(s2)
    ones_n = np.ones_like(s2h)
    ones_m = np.ones_like(t2h)

    lhs_rows, rhs_rows = [], []
    for d in range(3):
        # (sh+sm+sl)*(th+tm+tl): keep hh, hm, mh, hl, mm, lh cross terms
        lhs_rows += [sh[:, d], sh[:, d], sm[:, d], sh[:, d], sm[:, d], sl[:, d]]
        rhs_rows += [th[:, d], tm[:, d], th[:, d], tl[:, d], tm[:, d], th[:, d]]
    lhs_rows += [ones_n, ones_n, ones_n, s2h, s2m, s2l]
    rhs_rows += [t2h, t2m, t2l, ones_m, ones_m, ones_m]
    lhs24 = np.stack(lhs_rows, axis=1)  # [BPC, 24, N]
    rhs24 = np.stack(rhs_rows, axis=1)  # [BPC, 24, M]

    bpc = lhs24.shape[0]
    lhs = np.zeros((bpc, P, lhs24.shape[2]), dtype=BF16)
    rhs = np.zeros((bpc, P, MJ), dtype=BF16)
    for g in range(NG):
        lhs[:, 32 * g:32 * g + K, :] = lhs24
        rhs[:, 32 * g:32 * g + K, :] = rhs24[:, :, MJ * g:MJ * (g + 1)]
    return lhs, rhs


def kernel(input1, input2):
    global last_results
    a = np.asarray(input1, dtype=np.float32)  # [B, N, 3]
    b = np.asarray(input2, dtype=np.float32)  # [B, M, 3]
    assert a.shape == (B, N, D) and b.shape == (B, M, D)

    nc = _get_nc()
    in_maps = []
    for c in range(NCORES):
        sl = slice(c * BPC, (c + 1) * BPC)
        lhsA, rhsA = _pack(a[sl], b[sl])
        lhsB, rhsB = _pack(b[sl], a[sl])
        in_maps.append({"lhsA": lhsA, "rhsA": rhsA, "lhsB": lhsB, "rhsB": rhsB})

    r = run_bass_kernel_spmd(nc, in_maps, list(range(NCORES)), trace=trace)
    last_results = r

    total = 0.0
    for c in range(NCORES):
        mins = np.asarray(r.results[c]["mins"], dtype=np.float64)  # [2,BPC,P,NT]
        for bi in range(BPC):
            d0 = np.maximum(mins[0, bi].T.reshape(N), 0.0)  # index n = t*128 + p
            d1 = np.maximum(mins[1, bi].T.reshape(M), 0.0)
            total += max(np.sqrt(d0).mean(), np.sqrt(d1).mean())
    return np.float32(total / B)
